# revision 1
# baseline (speedup 1.0000x reference)
"""Trainium2 Bass kernel for KnowledgeEmbeddings (ragged_sequence).

Contract: kernel(**inputs) takes FULL unsharded inputs (numpy), returns the
FULL [64, 320, 768] f32 output.  Internally shards batch rows over 8
NeuronCores (8 rows each), replicates embedding tables, and runs a Tile/Bass
kernel per core via run_bass_kernel_spmd.

V2: table gather accumulates into the word-emb gather via DMA CCE add
(no DVE add); LN statistics via ACT accum_out passes (Square + Copy);
per-[128,1] stat math batched per group of 4 tiles; kvalid mask folded
into rstd.
"""

import functools
import numpy as np

import concourse.bass as bass
import concourse.tile as tile
from concourse import bacc, mybir
from concourse.bass import IndirectOffsetOnAxis
from concourse.bass_utils import run_bass_kernel_spmd
from concourse.masks import make_identity

# Problem constants (hardcoded per spec nn_KnowledgeEmbeddings_80839874445880)
WORD_LEN = 256
KN_LEN = 64
VOCAB = 30522
N_ENT = 500000
HID = 768
MAX_POS = 512
N_TYPES = 2
D_ENT = 100
B = 64
SEQ = WORD_LEN + KN_LEN  # 320
EPS = 1e-12

NCORES = 8
ROWS = B // NCORES           # 8 batch rows per core
WT = ROWS * WORD_LEN // 128  # 16 word tiles per core
KT = ROWS * KN_LEN // 128    # 4 knowledge tiles per core
NIDX = 2 * WT + 2 * KT       # idx tensor columns
GRP = 4                      # tiles per stats group

f32 = mybir.dt.float32
i32 = mybir.dt.int32
AF = mybir.ActivationFunctionType
ALU = mybir.AluOpType


# ---------------------------------------------------------------- host side

def _compact(ids: np.ndarray, tts: np.ndarray):
    """Vectorized numpy mirror of reference._compact_row."""
    ids = ids.astype(np.int64)
    wmask = (ids > 0) & (ids < VOCAB)
    worder = np.argsort(~wmask, axis=1, kind="stable")[:, :WORD_LEN]
    nw = wmask.sum(1, keepdims=True)
    wvalid = np.arange(WORD_LEN)[None, :] < nw
    wid = np.where(wvalid, np.take_along_axis(ids, worder, 1), 0)
    wtt = np.where(wvalid, np.take_along_axis(tts, worder, 1), 1)
    wpos = np.where(wvalid, worder, np.arange(WORD_LEN)[None, :])

    kmask = ids >= VOCAB
    korder = np.argsort(~kmask, axis=1, kind="stable")[:, :KN_LEN]
    nk = kmask.sum(1, keepdims=True)
    kvalid = np.arange(KN_LEN)[None, :] < nk
    kid = np.where(kvalid, np.take_along_axis(ids, korder, 1) - VOCAB, 0)
    ktt = np.where(kvalid, np.take_along_axis(tts, korder, 1), 0)
    kpos = np.where(kvalid, korder, 0)
    return wid, wtt, wpos, kid, ktt, kpos, kvalid


# ------------------------------------------------------------- device side

def _gather(nc, out_ap, table_ap, idx_col, accumulate=False):
    nc.gpsimd.indirect_dma_start(
        out=out_ap, out_offset=None, in_=table_ap,
        in_offset=IndirectOffsetOnAxis(ap=idx_col, axis=0),
        compute_op=ALU.add if accumulate else ALU.bypass,
    )


def _stats(nc, pools, X, SS_col, SM_col):
    """ACT passes: SS_col <- sum(X^2), SM_col <- sum(X) (per partition)."""
    scr = pools["scr"].tile([128, HID], f32, tag="scr")
    nc.scalar.activation(scr[:], X, func=AF.Square, accum_out=SS_col)
    scr2 = pools["scr"].tile([128, HID], f32, tag="scr")
    nc.scalar.activation(scr2[:], X, func=AF.Copy, accum_out=SM_col)


def _finish_stats(nc, pools, SS, SM, n, kv=None):
    """Batched [128, n] stat math.  Returns (U, RSTD) tiles.

    U = SM/HID;  RSTD = 1/sqrt(SS/HID - U^2 + eps)  (times kv if given).
    """
    spool = pools["small"]
    U_t = spool.tile([128, GRP], f32, tag="U")
    U = U_t[:, :n]
    nc.scalar.mul(U, SM, 1.0 / HID)
    SSs_t = spool.tile([128, GRP], f32, tag="SSs")
    SSs = SSs_t[:, :n]
    nc.scalar.mul(SSs, SS, 1.0 / HID)
    USQ_t = spool.tile([128, GRP], f32, tag="USQ")
    USQ = USQ_t[:, :n]
    nc.vector.tensor_mul(USQ, U, U)
    VAR_t = spool.tile([128, GRP], f32, tag="VAR")
    VAR = VAR_t[:, :n]
    nc.vector.tensor_tensor(out=VAR, in0=SSs, in1=USQ, op=ALU.subtract)
    RSTD_t = spool.tile([128, GRP], f32, tag="RSTD")
    RSTD = RSTD_t[:, :n]
    nc.scalar.activation(RSTD, VAR, func=AF.Sqrt, bias=pools["eps"][:])
    nc.vector.reciprocal(RSTD, RSTD)
    if kv is not None:
        nc.vector.tensor_mul(RSTD, RSTD, kv)
    return U, RSTD


def _normalize(nc, X, u_col, rstd_col, gamma_b, beta_b):
    nc.vector.tensor_scalar(
        out=X, in0=X, scalar1=u_col, scalar2=rstd_col,
        op0=ALU.subtract, op1=ALU.mult,
    )
    nc.vector.tensor_mul(X, X, gamma_b)
    nc.vector.tensor_add(X, X, beta_b)


def _device_kernel(tc, aps):
    nc = tc.nc
    we, ev, tbl, kwT, gb, idx, kvf, out = (
        aps["word_emb"], aps["entity_vec"], aps["table2"], aps["ke_wT"],
        aps["gamma_beta"], aps["idx"], aps["kvalid"], aps["out"],
    )
    import contextlib
    with contextlib.ExitStack() as ctx:
        singles = ctx.enter_context(tc.tile_pool(name="singles", bufs=1))
        xpool = ctx.enter_context(tc.tile_pool(name="x", bufs=2 * GRP + 2))
        spool = ctx.enter_context(tc.tile_pool(name="small", bufs=3))
        scrpool = ctx.enter_context(tc.tile_pool(name="scr", bufs=2))
        evpool = ctx.enter_context(tc.tile_pool(name="ev", bufs=3))
        psum = ctx.enter_context(tc.tile_pool(name="psum", bufs=2, space="PSUM"))

        eps_sb = singles.tile([128, 1], f32)
        nc.vector.memset(eps_sb[:], EPS)
        pools = {"small": spool, "scr": scrpool, "eps": eps_sb}

        # --- setup (once per core) ---
        idx_sb = singles.tile([128, NIDX], i32)
        nc.sync.dma_start(idx_sb[:], idx)
        kv_sb = singles.tile([128, KT], f32)
        nc.sync.dma_start(kv_sb[:], kvf)
        kw_sb = singles.tile([128, HID], f32)
        nc.vector.memset(kw_sb[:], 0.0)
        nc.sync.dma_start(kw_sb[:D_ENT, :], kwT)
        ident = singles.tile([128, 128], f32)
        make_identity(nc, ident[:])
        gbb = singles.tile([128, 4, HID], f32)
        gb_bcast = bass.AP(tensor=gb.tensor, offset=gb.offset,
                           ap=[[0, 128]] + list(gb.ap))
        nc.gpsimd.dma_start(out=gbb[:], in_=gb_bcast)

        # --- word tiles, in groups of GRP ---
        for g0 in range(0, WT, GRP):
            n = min(GRP, WT - g0)
            SS = spool.tile([128, GRP], f32, tag="SS")
            SM = spool.tile([128, GRP], f32, tag="SM")
            Xs = []
            for i in range(n):
                t = g0 + i
                X = xpool.tile([128, HID], f32, tag="X")
                _gather(nc, X[:], we, idx_sb[:, t:t + 1])
                _gather(nc, X[:], tbl, idx_sb[:, WT + t:WT + t + 1],
                        accumulate=True)
                _stats(nc, pools, X[:], SS[:, i:i + 1], SM[:, i:i + 1])
                Xs.append(X)
            U, RSTD = _finish_stats(nc, pools, SS[:, :n], SM[:, :n], n)
            for i in range(n):
                t = g0 + i
                X = Xs[i]
                _normalize(nc, X[:], U[:, i:i + 1], RSTD[:, i:i + 1],
                           gbb[:, 0, :], gbb[:, 1, :])
                b, h = divmod(t, 2)
                r = b * SEQ + h * 128
                nc.sync.dma_start(out[r:r + 128, :], X[:])

        # --- knowledge tiles (one group of KT) ---
        SS = spool.tile([128, GRP], f32, tag="SS")
        SM = spool.tile([128, GRP], f32, tag="SM")
        Xs = []
        for c in range(KT):
            EVt = evpool.tile([128, D_ENT], f32, tag="EV")
            _gather(nc, EVt[:], ev, idx_sb[:, 2 * WT + c:2 * WT + c + 1])
            ps_t = psum.tile([D_ENT, 128], f32, tag="pst")
            nc.tensor.transpose(out=ps_t[:], in_=EVt[:], identity=ident[:])
            EVT = evpool.tile([128, 128], f32, tag="EVT")
            nc.vector.memset(EVT[:], 0.0)
            nc.scalar.copy(EVT[:D_ENT, :], ps_t[:])

            X = xpool.tile([128, HID], f32, tag="X")
            _gather(nc, X[:], tbl,
                    idx_sb[:, 2 * WT + KT + c:2 * WT + KT + c + 1])
            for half in range(2):
                pm = psum.tile([128, 384], f32, tag="mm")
                nc.tensor.matmul(
                    out=pm[:], lhsT=EVT[:],
                    rhs=kw_sb[:, 384 * half:384 * (half + 1)],
                    start=True, stop=True,
                )
                nc.vector.tensor_add(
                    X[:, 384 * half:384 * (half + 1)],
                    X[:, 384 * half:384 * (half + 1)], pm[:],
                )
            _stats(nc, pools, X[:], SS[:, c:c + 1], SM[:, c:c + 1])
            Xs.append(X)
        # rstd *= kvalid: pad rows then normalize to 0 -> output = k_beta
        U, RSTD = _finish_stats(nc, pools, SS[:, :KT], SM[:, :KT], KT,
                                kv=kv_sb[:])
        for c in range(KT):
            X = Xs[c]
            _normalize(nc, X[:], U[:, c:c + 1], RSTD[:, c:c + 1],
                       gbb[:, 2, :], gbb[:, 3, :])
            r0 = (2 * c) * SEQ + WORD_LEN
            r1 = (2 * c + 1) * SEQ + WORD_LEN
            nc.sync.dma_start(out[r0:r0 + 64, :], X[0:64, :])
            nc.sync.dma_start(out[r1:r1 + 64, :], X[64:128, :])


@functools.lru_cache(maxsize=1)
def build_program():
    nc = bacc.Bacc("TRN2", target_bir_lowering=False, debug=False,
                   enable_asserts=False)
    aps = {
        "word_emb": nc.dram_tensor("word_emb", [VOCAB, HID], f32,
                                   kind="ExternalInput").ap(),
        "entity_vec": nc.dram_tensor("entity_vec", [N_ENT, D_ENT], f32,
                                     kind="ExternalInput").ap(),
        "table2": nc.dram_tensor("table2", [2 * N_TYPES * MAX_POS, HID], f32,
                                 kind="ExternalInput").ap(),
        "ke_wT": nc.dram_tensor("ke_wT", [D_ENT, HID], f32,
                                kind="ExternalInput").ap(),
        "gamma_beta": nc.dram_tensor("gamma_beta", [4, HID], f32,
                                     kind="ExternalInput").ap(),
        "idx": nc.dram_tensor("idx", [128, NIDX], i32,
                              kind="ExternalInput").ap(),
        "kvalid": nc.dram_tensor("kvalid", [128, KT], f32,
                                 kind="ExternalInput").ap(),
        "out": nc.dram_tensor("out", [ROWS * SEQ, HID], f32,
                              kind="ExternalOutput").ap(),
    }
    with tile.TileContext(nc) as tc:
        _device_kernel(tc, aps)
    nc.compile()
    return nc


def _prepare_in_maps(inputs):
    input_ids = np.asarray(inputs["input_ids"], dtype=np.int32)
    token_type_ids = np.asarray(inputs["token_type_ids"], dtype=np.int32)
    word_emb = np.ascontiguousarray(np.asarray(inputs["word_emb"], np.float32))
    pos_emb = np.asarray(inputs["pos_emb"], np.float32)
    tt_emb = np.asarray(inputs["tt_emb"], np.float32)
    entity_vec = np.ascontiguousarray(np.asarray(inputs["entityVec"], np.float32))
    ke_w = np.asarray(inputs["ke_w"], np.float32)
    ke_b = np.asarray(inputs["ke_b"], np.float32)

    # fused side table: rows [tt*512 + pos] = pos_emb[pos] + tt_emb[tt],
    # second half additionally + ke_b (knowledge branch folds its bias in)
    base = (tt_emb[:, None, :] + pos_emb[None, :, :]).reshape(
        N_TYPES * MAX_POS, HID)
    table2 = np.ascontiguousarray(
        np.concatenate([base, base + ke_b[None, :]], axis=0))
    ke_wT = np.ascontiguousarray(ke_w.T)
    gamma_beta = np.ascontiguousarray(np.stack([
        np.asarray(inputs["w_gamma"], np.float32),
        np.asarray(inputs["w_beta"], np.float32),
        np.asarray(inputs["k_gamma"], np.float32),
        np.asarray(inputs["k_beta"], np.float32),
    ]))

    wid, wtt, wpos, kid, ktt, kpos, kvalid = _compact(input_ids, token_type_ids)
    widx = wid.astype(np.int32)
    wtidx = (wpos + MAX_POS * wtt).astype(np.int32)
    kidx = kid.astype(np.int32)
    ktidx = (N_TYPES * MAX_POS + kpos + MAX_POS * ktt).astype(np.int32)
    kvf = kvalid.astype(np.float32)

    in_maps = []
    for c in range(NCORES):
        s = slice(c * ROWS, (c + 1) * ROWS)
        idx = np.concatenate([
            widx[s].reshape(WT, 128).T,
            wtidx[s].reshape(WT, 128).T,
            kidx[s].reshape(KT, 128).T,
            ktidx[s].reshape(KT, 128).T,
        ], axis=1)
        in_maps.append({
            "word_emb": word_emb,
            "entity_vec": entity_vec,
            "table2": table2,
            "ke_wT": ke_wT,
            "gamma_beta": gamma_beta,
            "idx": np.ascontiguousarray(idx),
            "kvalid": np.ascontiguousarray(kvf[s].reshape(KT, 128).T),
        })
    return in_maps


def run(inputs, trace=False):
    """Returns (full_output [64,320,768] f32, exec_time_ns or None)."""
    nc = build_program()
    in_maps = _prepare_in_maps(inputs)
    res = run_bass_kernel_spmd(nc, in_maps, list(range(NCORES)), trace=trace)
    out = np.concatenate(
        [r["out"].reshape(ROWS, SEQ, HID) for r in res.results], axis=0)
    return out, res.exec_time_ns


def kernel(**inputs) -> np.ndarray:
    out, _ = run(inputs)
    return out



# revision 10
# speedup vs baseline: 1.1794x; 1.1794x over previous
"""Trainium2 Bass kernel for KnowledgeEmbeddings (ragged_sequence).

Contract: kernel(**inputs) takes FULL unsharded inputs (numpy), returns the
FULL [64, 320, 768] f32 output.  Internally shards batch rows over 8
NeuronCores (8 rows each), replicates embedding tables, and runs a Tile/Bass
kernel per core via run_bass_kernel_spmd.

V3: all tables + on-chip tensors in bf16 (halves HBM traffic; output upcast
to f32 on host); indirect gathers batched 4 tiles per instruction via
[128, 4] offset APs (cuts SWDGE descriptor-generation serialization ~4x);
LN statistics via DVE bn_stats/bn_aggr instead of two ACT passes; output
writes batched 4 tiles per DMA with strided DRAM APs.
"""

import functools
import numpy as np
import ml_dtypes

import concourse.bass as bass
import concourse.tile as tile
from concourse import bacc, mybir
from concourse.bass import IndirectOffsetOnAxis
from concourse.bass_utils import run_bass_kernel_spmd
from concourse.masks import make_identity

# Problem constants (hardcoded per spec nn_KnowledgeEmbeddings_80839874445880)
WORD_LEN = 256
KN_LEN = 64
VOCAB = 30522
N_ENT = 500000
HID = 768
MAX_POS = 512
N_TYPES = 2
D_ENT = 100
B = 64
SEQ = WORD_LEN + KN_LEN  # 320
EPS = 1e-12

NCORES = 8
ROWS = B // NCORES           # 8 batch rows per core
WT = ROWS * WORD_LEN // 128  # 16 word tiles per core
KT = ROWS * KN_LEN // 128    # 4 knowledge tiles per core
NIDX = 2 * WT + 2 * KT       # idx tensor columns
GRP = 4                      # tiles per gather/stats group
MULTIROW_GATHER = False      # one indirect DMA per GRP tiles vs per tile

f32 = mybir.dt.float32
bf16 = mybir.dt.bfloat16
i32 = mybir.dt.int32
AF = mybir.ActivationFunctionType
ALU = mybir.AluOpType

BF16NP = np.dtype(ml_dtypes.bfloat16)


# ---------------------------------------------------------------- host side

def _compact(ids: np.ndarray, tts: np.ndarray):
    """Vectorized numpy mirror of reference._compact_row."""
    ids = ids.astype(np.int64)
    wmask = (ids > 0) & (ids < VOCAB)
    worder = np.argsort(~wmask, axis=1, kind="stable")[:, :WORD_LEN]
    nw = wmask.sum(1, keepdims=True)
    wvalid = np.arange(WORD_LEN)[None, :] < nw
    wid = np.where(wvalid, np.take_along_axis(ids, worder, 1), 0)
    wtt = np.where(wvalid, np.take_along_axis(tts, worder, 1), 1)
    wpos = np.where(wvalid, worder, np.arange(WORD_LEN)[None, :])

    kmask = ids >= VOCAB
    korder = np.argsort(~kmask, axis=1, kind="stable")[:, :KN_LEN]
    nk = kmask.sum(1, keepdims=True)
    kvalid = np.arange(KN_LEN)[None, :] < nk
    kid = np.where(kvalid, np.take_along_axis(ids, korder, 1) - VOCAB, 0)
    ktt = np.where(kvalid, np.take_along_axis(tts, korder, 1), 0)
    kpos = np.where(kvalid, korder, 0)
    return wid, wtt, wpos, kid, ktt, kpos, kvalid


# ------------------------------------------------------------- device side

def _gather(nc, out_ap, table_ap, idx_ap, accumulate=False):
    nc.gpsimd.indirect_dma_start(
        out=out_ap, out_offset=None, in_=table_ap,
        in_offset=IndirectOffsetOnAxis(ap=idx_ap, axis=0),
        compute_op=ALU.add if accumulate else ALU.bypass,
    )


def _reshaped(ap, dims):
    """View a contiguous SBUF AP [128, ...] as [128, *dims]."""
    new = [list(ap.ap[0])]
    sz = 1
    for d in dims:
        sz *= d
    assert sz == np.prod([d[1] for d in ap.ap[1:]])
    stride = 1
    rev = []
    for d in reversed(dims):
        rev.append([stride, d])
        stride *= d
    new.extend(reversed(rev))
    return bass.AP(tensor=ap.tensor, offset=ap.offset, ap=new)


def _device_kernel(tc, aps):
    nc = tc.nc
    we, ev, tbl, kwT, gb, idx, kvf, out = (
        aps["word_emb"], aps["entity_vec"], aps["table2"], aps["ke_wT"],
        aps["gamma_beta"], aps["idx"], aps["kvalid"], aps["out"],
    )
    import contextlib
    with contextlib.ExitStack() as ctx:
        singles = ctx.enter_context(tc.tile_pool(name="singles", bufs=1))
        xpool = ctx.enter_context(tc.tile_pool(name="x", bufs=3))
        evpool = ctx.enter_context(tc.tile_pool(name="ev", bufs=2))
        spool = ctx.enter_context(tc.tile_pool(name="small", bufs=3))
        psum = ctx.enter_context(tc.tile_pool(name="psum", bufs=2, space="PSUM"))

        eps_sb = singles.tile([128, 1], f32)
        nc.vector.memset(eps_sb[:], EPS)

        # --- setup (once per core) ---
        idx_sb = singles.tile([128, NIDX], i32)
        nc.sync.dma_start(idx_sb[:], idx)
        kv_sb = singles.tile([128, KT], f32)
        nc.sync.dma_start(kv_sb[:], kvf)
        kw_sb = singles.tile([128, HID], bf16)
        nc.vector.memset(kw_sb[:], 0.0)
        nc.sync.dma_start(kw_sb[:D_ENT, :], kwT)
        ident = singles.tile([128, 128], bf16)
        make_identity(nc, ident[:])
        gbb = singles.tile([128, 4, HID], bf16)
        gb_bcast = bass.AP(tensor=gb.tensor, offset=gb.offset,
                           ap=[[0, 128]] + list(gb.ap))
        nc.gpsimd.dma_start(out=gbb[:], in_=gb_bcast)
        # two persistent transposed-EV buffers; rows D_ENT..127 stay zero
        EVTs = [singles.tile([128, 128], bf16, name=f"EVT{i}")
                for i in range(2)]
        for e in EVTs:
            nc.vector.memset(e[:], 0.0)

        def gb_row_bcast(row, n):
            g = gbb[:, row, :]
            return bass.AP(tensor=g.tensor, offset=g.offset,
                           ap=[list(g.ap[0]), [0, n], list(g.ap[1])])

        def ln_group(X, n, gi, kv=None):
            """In-place LN of X [128, n, HID] (bf16): gamma/beta rows 2*gi/2*gi+1.

            If kv given ([128, n] f32 0/1 mask), rstd *= kv (pad rows -> beta).
            """
            SS_t = spool.tile([128, GRP], f32, tag="SS")
            SS = SS_t[:, :n]
            SM_t = spool.tile([128, GRP], f32, tag="SM")
            SM = SM_t[:, :n]
            scr = spool.tile([128, HID], bf16, tag="scr")
            for i in range(n):
                nc.scalar.activation(scr[:], X[:, i, :], func=AF.Square,
                                     accum_out=SS_t[:, i:i + 1])
            nc.vector.reduce_sum(SM, X[:], axis=mybir.AxisListType.X)
            # u = SM/HID; rstd = 1/sqrt(SS/HID - u^2 + eps)
            U_t = spool.tile([128, GRP], f32, tag="U")
            U = U_t[:, :n]
            nc.vector.tensor_scalar_mul(U, SM, 1.0 / HID)
            VAR_t = spool.tile([128, GRP], f32, tag="VAR")
            VAR = VAR_t[:, :n]
            nc.vector.tensor_scalar_mul(VAR, SS, 1.0 / HID)
            USQ_t = spool.tile([128, GRP], f32, tag="USQ")
            USQ = USQ_t[:, :n]
            nc.vector.tensor_mul(USQ, U, U)
            nc.vector.tensor_tensor(out=VAR, in0=VAR, in1=USQ,
                                    op=ALU.subtract)
            RS_t = spool.tile([128, GRP], f32, tag="RS")
            RS = RS_t[:, :n]
            nc.scalar.activation(RS, VAR, func=AF.Sqrt, bias=eps_sb[:])
            nc.vector.reciprocal(RS, RS)
            if kv is not None:
                nc.vector.tensor_mul(RS, RS, kv)
            for i in range(n):
                nc.vector.tensor_scalar(
                    out=X[:, i, :], in0=X[:, i, :],
                    scalar1=U_t[:, i:i + 1], scalar2=RS_t[:, i:i + 1],
                    op0=ALU.subtract, op1=ALU.mult,
                )
            nc.vector.tensor_mul(X[:], X[:], gb_row_bcast(2 * gi, n))
            nc.vector.tensor_add(X[:], X[:], gb_row_bcast(2 * gi + 1, n))

        # --- word tiles, in groups of GRP ---
        for g in range(WT // GRP):
            t0 = g * GRP
            X = xpool.tile([128, GRP, HID], bf16, tag="X")
            if MULTIROW_GATHER:
                _gather(nc, X[:], we, idx_sb[:, t0:t0 + GRP])
                _gather(nc, X[:], tbl, idx_sb[:, WT + t0:WT + t0 + GRP],
                        accumulate=True)
            else:
                for i in range(GRP):
                    t = t0 + i
                    _gather(nc, X[:, i, :], we, idx_sb[:, t:t + 1])
                    _gather(nc, X[:, i, :], tbl, idx_sb[:, WT + t:WT + t + 1],
                            accumulate=True)
            ln_group(X, GRP, 0)
            # tiles t0..t0+3 = batch rows 2g, 2g+1, halves h=0/1
            for b in range(2):
                wout = bass.AP(
                    tensor=out.tensor, offset=(2 * g + b) * SEQ * HID,
                    ap=[[HID, 128], [128 * HID, 2], [1, HID]])
                nc.sync.dma_start(wout, X[:, 2 * b:2 * b + 2, :])

        # --- knowledge tiles (one group of KT) ---
        EV = evpool.tile([128, KT, D_ENT], bf16, tag="EV")
        Xk = xpool.tile([128, KT, HID], bf16, tag="X")
        if MULTIROW_GATHER:
            _gather(nc, EV[:], ev, idx_sb[:, 2 * WT:2 * WT + KT])
            _gather(nc, Xk[:], tbl, idx_sb[:, 2 * WT + KT:2 * WT + 2 * KT])
        else:
            for c in range(KT):
                _gather(nc, EV[:, c, :], ev,
                        idx_sb[:, 2 * WT + c:2 * WT + c + 1])
                _gather(nc, Xk[:, c, :], tbl,
                        idx_sb[:, 2 * WT + KT + c:2 * WT + KT + c + 1])
        for c in range(KT):
            ps_t = psum.tile([D_ENT, 128], bf16, tag="pst")
            nc.tensor.transpose(out=ps_t[:], in_=EV[:, c, :], identity=ident[:])
            EVT = EVTs[c % 2]
            nc.scalar.copy(EVT[:D_ENT, :], ps_t[:])
            for half in range(2):
                pm = psum.tile([128, 384], f32, tag="mm")
                nc.tensor.matmul(
                    out=pm[:], lhsT=EVT[:],
                    rhs=kw_sb[:, 384 * half:384 * (half + 1)],
                    start=True, stop=True,
                )
                sl = slice(384 * half, 384 * (half + 1))
                nc.vector.tensor_add(Xk[:, c, sl], Xk[:, c, sl], pm[:])
        # rstd *= kvalid: pad rows then normalize to 0 -> output = k_beta
        ln_group(Xk, KT, 1, kv=kv_sb[:])
        for b2 in range(2):
            kout = bass.AP(
                tensor=out.tensor, offset=(WORD_LEN + b2 * SEQ) * HID,
                ap=[[HID, 64], [2 * SEQ * HID, KT], [1, HID]])
            nc.sync.dma_start(kout, Xk[64 * b2:64 * (b2 + 1), :, :])


@functools.lru_cache(maxsize=1)
def build_program():
    nc = bacc.Bacc("TRN2", target_bir_lowering=False, debug=False,
                   enable_asserts=False)
    aps = {
        "word_emb": nc.dram_tensor("word_emb", [VOCAB, HID], bf16,
                                   kind="ExternalInput").ap(),
        "entity_vec": nc.dram_tensor("entity_vec", [N_ENT, D_ENT], bf16,
                                     kind="ExternalInput").ap(),
        "table2": nc.dram_tensor("table2", [2 * N_TYPES * MAX_POS, HID], bf16,
                                 kind="ExternalInput").ap(),
        "ke_wT": nc.dram_tensor("ke_wT", [D_ENT, HID], bf16,
                                kind="ExternalInput").ap(),
        "gamma_beta": nc.dram_tensor("gamma_beta", [4, HID], bf16,
                                     kind="ExternalInput").ap(),
        "idx": nc.dram_tensor("idx", [128, NIDX], i32,
                              kind="ExternalInput").ap(),
        "kvalid": nc.dram_tensor("kvalid", [128, KT], f32,
                                 kind="ExternalInput").ap(),
        "out": nc.dram_tensor("out", [ROWS * SEQ, HID], bf16,
                              kind="ExternalOutput").ap(),
    }
    with tile.TileContext(nc) as tc:
        _device_kernel(tc, aps)
    nc.compile()
    return nc


def _prepare_in_maps(inputs):
    input_ids = np.asarray(inputs["input_ids"], dtype=np.int32)
    token_type_ids = np.asarray(inputs["token_type_ids"], dtype=np.int32)
    word_emb = np.ascontiguousarray(
        np.asarray(inputs["word_emb"], np.float32).astype(BF16NP))
    pos_emb = np.asarray(inputs["pos_emb"], np.float32)
    tt_emb = np.asarray(inputs["tt_emb"], np.float32)
    entity_vec = np.ascontiguousarray(
        np.asarray(inputs["entityVec"], np.float32).astype(BF16NP))
    ke_w = np.asarray(inputs["ke_w"], np.float32)
    ke_b = np.asarray(inputs["ke_b"], np.float32)

    # fused side table: rows [tt*512 + pos] = pos_emb[pos] + tt_emb[tt],
    # second half additionally + ke_b (knowledge branch folds its bias in)
    base = (tt_emb[:, None, :] + pos_emb[None, :, :]).reshape(
        N_TYPES * MAX_POS, HID)
    table2 = np.ascontiguousarray(
        np.concatenate([base, base + ke_b[None, :]], axis=0).astype(BF16NP))
    ke_wT = np.ascontiguousarray(ke_w.T.astype(BF16NP))
    gamma_beta = np.ascontiguousarray(np.stack([
        np.asarray(inputs["w_gamma"], np.float32),
        np.asarray(inputs["w_beta"], np.float32),
        np.asarray(inputs["k_gamma"], np.float32),
        np.asarray(inputs["k_beta"], np.float32),
    ]).astype(BF16NP))

    wid, wtt, wpos, kid, ktt, kpos, kvalid = _compact(input_ids, token_type_ids)
    widx = wid.astype(np.int32)
    wtidx = (wpos + MAX_POS * wtt).astype(np.int32)
    kidx = kid.astype(np.int32)
    ktidx = (N_TYPES * MAX_POS + kpos + MAX_POS * ktt).astype(np.int32)
    kvf = kvalid.astype(np.float32)

    in_maps = []
    for c in range(NCORES):
        s = slice(c * ROWS, (c + 1) * ROWS)
        idx = np.concatenate([
            widx[s].reshape(WT, 128).T,
            wtidx[s].reshape(WT, 128).T,
            kidx[s].reshape(KT, 128).T,
            ktidx[s].reshape(KT, 128).T,
        ], axis=1)
        in_maps.append({
            "word_emb": word_emb,
            "entity_vec": entity_vec,
            "table2": table2,
            "ke_wT": ke_wT,
            "gamma_beta": gamma_beta,
            "idx": np.ascontiguousarray(idx),
            "kvalid": np.ascontiguousarray(kvf[s].reshape(KT, 128).T),
        })
    return in_maps


def run(inputs, trace=False):
    """Returns (full_output [64,320,768] f32, exec_time_ns or None)."""
    nc = build_program()
    in_maps = _prepare_in_maps(inputs)
    res = run_bass_kernel_spmd(nc, in_maps, list(range(NCORES)), trace=trace)
    out = np.concatenate(
        [np.asarray(r["out"]).astype(np.float32).reshape(ROWS, SEQ, HID)
         for r in res.results], axis=0)
    return out, res.exec_time_ns


def kernel(**inputs) -> np.ndarray:
    out, _ = run(inputs)
    return out


# revision 12
# speedup vs baseline: 1.2605x; 1.0687x over previous
"""Trainium2 Bass kernel for KnowledgeEmbeddings (ragged_sequence).

Contract: kernel(**inputs) takes FULL unsharded inputs (numpy), returns the
FULL [64, 320, 768] f32 output.  Internally shards batch rows over 8
NeuronCores (8 rows each), replicates embedding tables, and runs a Tile/Bass
kernel per core via run_bass_kernel_spmd.

V4: all tables + on-chip tensors in bf16 (halves HBM traffic; output upcast
to f32 on host); word/table2 gathers batched 512 rows per instruction via
the dma_gather ucode op (int16 indices) instead of 128-row indirect DMAs --
cuts SWDGE descriptor-generation serialization ~4x; the table2 add is fused
with the per-tile row-sum via scalar_tensor_tensor(accum_out); LN sum-of-
squares on ACT (Square+accum); batched strided output writes.
"""

import functools
import numpy as np
import ml_dtypes

import concourse.bass as bass
import concourse.tile as tile
from concourse import bacc, mybir
from concourse.bass import IndirectOffsetOnAxis
from concourse.bass_utils import run_bass_kernel_spmd
from concourse.masks import make_identity

# Problem constants (hardcoded per spec nn_KnowledgeEmbeddings_80839874445880)
WORD_LEN = 256
KN_LEN = 64
VOCAB = 30522
N_ENT = 500000
HID = 768
MAX_POS = 512
N_TYPES = 2
D_ENT = 100
B = 64
SEQ = WORD_LEN + KN_LEN  # 320
EPS = 1e-12

NCORES = 8
ROWS = B // NCORES           # 8 batch rows per core
WT = ROWS * WORD_LEN // 128  # 16 word tiles per core
KT = ROWS * KN_LEN // 128    # 4 knowledge tiles per core
GRP = 4                      # tiles per gather/stats group
NG = WT // GRP               # word groups
GI = GRP * 128 // 16         # idx16 columns per group (32)
NI16 = (2 * NG + 1) * GI     # idx16 tensor columns (9 gathers x 32)

f32 = mybir.dt.float32
bf16 = mybir.dt.bfloat16
i32 = mybir.dt.int32
i16 = mybir.dt.int16
AF = mybir.ActivationFunctionType
ALU = mybir.AluOpType

BF16NP = np.dtype(ml_dtypes.bfloat16)


# ---------------------------------------------------------------- host side

def _compact(ids: np.ndarray, tts: np.ndarray):
    """Vectorized numpy mirror of reference._compact_row."""
    ids = ids.astype(np.int64)
    wmask = (ids > 0) & (ids < VOCAB)
    worder = np.argsort(~wmask, axis=1, kind="stable")[:, :WORD_LEN]
    nw = wmask.sum(1, keepdims=True)
    wvalid = np.arange(WORD_LEN)[None, :] < nw
    wid = np.where(wvalid, np.take_along_axis(ids, worder, 1), 0)
    wtt = np.where(wvalid, np.take_along_axis(tts, worder, 1), 1)
    wpos = np.where(wvalid, worder, np.arange(WORD_LEN)[None, :])

    kmask = ids >= VOCAB
    korder = np.argsort(~kmask, axis=1, kind="stable")[:, :KN_LEN]
    nk = kmask.sum(1, keepdims=True)
    kvalid = np.arange(KN_LEN)[None, :] < nk
    kid = np.where(kvalid, np.take_along_axis(ids, korder, 1) - VOCAB, 0)
    ktt = np.where(kvalid, np.take_along_axis(tts, korder, 1), 0)
    kpos = np.where(kvalid, korder, 0)
    return wid, wtt, wpos, kid, ktt, kpos, kvalid


def _wrap16(flat: np.ndarray) -> np.ndarray:
    """[n] int16 -> [128, n//16] dma_gather index layout (value i at
    [i%16, i//16], replicated across the 8 Q7 16-partition blocks)."""
    n = flat.shape[0]
    w = flat.reshape(n // 16, 16).T  # [16, n//16]
    return np.tile(w, (8, 1))


# ------------------------------------------------------------- device side

def _device_kernel(tc, aps):
    nc = tc.nc
    we, ev, tbl, kwT, gb, idx16, idxk, kvf, out = (
        aps["word_emb"], aps["entity_vec"], aps["table2"], aps["ke_wT"],
        aps["gamma_beta"], aps["idx16"], aps["idxk"], aps["kvalid"],
        aps["out"],
    )
    import contextlib
    with contextlib.ExitStack() as ctx:
        singles = ctx.enter_context(tc.tile_pool(name="singles", bufs=1))
        xpool = ctx.enter_context(tc.tile_pool(name="x", bufs=3))
        tpool = ctx.enter_context(tc.tile_pool(name="t", bufs=3))
        evpool = ctx.enter_context(tc.tile_pool(name="ev", bufs=2))
        spool = ctx.enter_context(tc.tile_pool(name="small", bufs=3))
        psum = ctx.enter_context(tc.tile_pool(name="psum", bufs=2, space="PSUM"))

        eps_sb = singles.tile([128, 1], f32)
        nc.vector.memset(eps_sb[:], EPS)

        # --- setup (once per core) ---
        idx16_sb = singles.tile([128, NI16], i16)
        nc.sync.dma_start(idx16_sb[:], idx16)
        idxk_sb = singles.tile([128, KT], i32)
        nc.sync.dma_start(idxk_sb[:], idxk)
        kv_sb = singles.tile([128, KT], f32)
        nc.sync.dma_start(kv_sb[:], kvf)
        kw_sb = singles.tile([128, HID], bf16)
        nc.vector.memset(kw_sb[:], 0.0)
        nc.sync.dma_start(kw_sb[:D_ENT, :], kwT)
        ident = singles.tile([128, 128], bf16)
        make_identity(nc, ident[:])
        gbb = singles.tile([128, 4, HID], bf16)
        gb_bcast = bass.AP(tensor=gb.tensor, offset=gb.offset,
                           ap=[[0, 128]] + list(gb.ap))
        nc.gpsimd.dma_start(out=gbb[:], in_=gb_bcast)
        # two persistent transposed-EV buffers; rows D_ENT..127 stay zero
        EVTs = [singles.tile([128, 128], bf16, name=f"EVT{i}")
                for i in range(2)]
        for e in EVTs:
            nc.vector.memset(e[:], 0.0)

        def gb_row_bcast(row, n):
            g = gbb[:, row, :]
            return bass.AP(tensor=g.tensor, offset=g.offset,
                           ap=[list(g.ap[0]), [0, n], list(g.ap[1])])

        def ln_finish(X, n, gi, SS_t, SM_t, kv=None):
            """Normalize X [128, n, HID] in place given per-tile sum (SM) and
            sum-of-squares (SS) columns; apply gamma/beta rows 2gi/2gi+1."""
            U_t = spool.tile([128, GRP], f32, tag="U")
            U = U_t[:, :n]
            nc.vector.tensor_scalar_mul(U, SM_t[:, :n], 1.0 / HID)
            VAR_t = spool.tile([128, GRP], f32, tag="VAR")
            VAR = VAR_t[:, :n]
            nc.vector.tensor_scalar_mul(VAR, SS_t[:, :n], 1.0 / HID)
            USQ_t = spool.tile([128, GRP], f32, tag="USQ")
            USQ = USQ_t[:, :n]
            nc.vector.tensor_mul(USQ, U, U)
            nc.vector.tensor_tensor(out=VAR, in0=VAR, in1=USQ,
                                    op=ALU.subtract)
            RS_t = spool.tile([128, GRP], f32, tag="RS")
            RS = RS_t[:, :n]
            nc.scalar.activation(RS, VAR, func=AF.Sqrt, bias=eps_sb[:])
            nc.vector.reciprocal(RS, RS)
            if kv is not None:
                nc.vector.tensor_mul(RS, RS, kv)
            for i in range(n):
                nc.vector.tensor_scalar(
                    out=X[:, i, :], in0=X[:, i, :],
                    scalar1=U_t[:, i:i + 1], scalar2=RS_t[:, i:i + 1],
                    op0=ALU.subtract, op1=ALU.mult,
                )
            nc.vector.tensor_mul(X[:], X[:], gb_row_bcast(2 * gi, n))
            nc.vector.tensor_add(X[:], X[:], gb_row_bcast(2 * gi + 1, n))

        # --- word tiles, in groups of GRP ---
        for g in range(NG):
            A = xpool.tile([128, GRP, HID], bf16, tag="A")
            nc.gpsimd.dma_gather(
                A[:], we, idx16_sb[:, g * GI:(g + 1) * GI],
                GRP * 128, GRP * 128, HID)
            T = tpool.tile([128, GRP, HID], bf16, tag="T")
            nc.gpsimd.dma_gather(
                T[:], tbl, idx16_sb[:, (NG + g) * GI:(NG + g + 1) * GI],
                GRP * 128, GRP * 128, HID)
            SS_t = spool.tile([128, GRP], f32, tag="SS")
            SM_t = spool.tile([128, GRP], f32, tag="SM")
            scr = spool.tile([128, HID], bf16, tag="scr")
            for i in range(GRP):
                # A[i] += T[i], accumulating the row sum as a side effect
                nc.vector.scalar_tensor_tensor(
                    out=A[:, i, :], in0=A[:, i, :], scalar=0.0,
                    in1=T[:, i, :], op0=ALU.add, op1=ALU.add,
                    accum_out=SM_t[:, i:i + 1])
                nc.scalar.activation(scr[:], A[:, i, :], func=AF.Square,
                                     accum_out=SS_t[:, i:i + 1])
            ln_finish(A, GRP, 0, SS_t, SM_t)
            # tiles 4g..4g+3 = batch rows 2g, 2g+1, halves h=0/1
            for b in range(2):
                wout = bass.AP(
                    tensor=out.tensor, offset=(2 * g + b) * SEQ * HID,
                    ap=[[HID, 128], [128 * HID, 2], [1, HID]])
                nc.sync.dma_start(wout, A[:, 2 * b:2 * b + 2, :])

        # --- knowledge tiles (one group of KT) ---
        EV = evpool.tile([128, KT, D_ENT], bf16, tag="EV")
        for c in range(KT):
            nc.gpsimd.indirect_dma_start(
                out=EV[:, c, :], out_offset=None, in_=ev,
                in_offset=IndirectOffsetOnAxis(ap=idxk_sb[:, c:c + 1], axis=0),
            )
        Xk = xpool.tile([128, KT, HID], bf16, tag="A")
        nc.gpsimd.dma_gather(
            Xk[:], tbl, idx16_sb[:, 2 * NG * GI:(2 * NG + 1) * GI],
            KT * 128, KT * 128, HID)
        SS_t = spool.tile([128, GRP], f32, tag="SS")
        SM_t = spool.tile([128, GRP], f32, tag="SM")
        scr = spool.tile([128, HID], bf16, tag="scr")
        for c in range(KT):
            ps_t = psum.tile([D_ENT, 128], bf16, tag="pst")
            nc.tensor.transpose(out=ps_t[:], in_=EV[:, c, :], identity=ident[:])
            EVT = EVTs[c % 2]
            nc.scalar.copy(EVT[:D_ENT, :], ps_t[:])
            for half in range(2):
                pm = psum.tile([128, 384], f32, tag="mm")
                nc.tensor.matmul(
                    out=pm[:], lhsT=EVT[:],
                    rhs=kw_sb[:, 384 * half:384 * (half + 1)],
                    start=True, stop=True,
                )
                sl = slice(384 * half, 384 * (half + 1))
                nc.vector.tensor_add(Xk[:, c, sl], Xk[:, c, sl], pm[:])
            # identity pass for the row sum; Square pass for sum-of-squares
            nc.vector.tensor_scalar(
                out=Xk[:, c, :], in0=Xk[:, c, :], scalar1=0.0, scalar2=0.0,
                op0=ALU.add, op1=ALU.add, accum_out=SM_t[:, c:c + 1])
            nc.scalar.activation(scr[:], Xk[:, c, :], func=AF.Square,
                                 accum_out=SS_t[:, c:c + 1])
        # rstd *= kvalid: pad rows then normalize to 0 -> output = k_beta
        ln_finish(Xk, KT, 1, SS_t, SM_t, kv=kv_sb[:])
        for b2 in range(2):
            kout = bass.AP(
                tensor=out.tensor, offset=(WORD_LEN + b2 * SEQ) * HID,
                ap=[[HID, 64], [2 * SEQ * HID, KT], [1, HID]])
            nc.sync.dma_start(kout, Xk[64 * b2:64 * (b2 + 1), :, :])


@functools.lru_cache(maxsize=1)
def build_program():
    nc = bacc.Bacc("TRN2", target_bir_lowering=False, debug=False,
                   enable_asserts=False)
    aps = {
        "word_emb": nc.dram_tensor("word_emb", [VOCAB, HID], bf16,
                                   kind="ExternalInput").ap(),
        "entity_vec": nc.dram_tensor("entity_vec", [N_ENT, D_ENT], bf16,
                                     kind="ExternalInput").ap(),
        "table2": nc.dram_tensor("table2", [2 * N_TYPES * MAX_POS, HID], bf16,
                                 kind="ExternalInput").ap(),
        "ke_wT": nc.dram_tensor("ke_wT", [D_ENT, HID], bf16,
                                kind="ExternalInput").ap(),
        "gamma_beta": nc.dram_tensor("gamma_beta", [4, HID], bf16,
                                     kind="ExternalInput").ap(),
        "idx16": nc.dram_tensor("idx16", [128, NI16], i16,
                                kind="ExternalInput").ap(),
        "idxk": nc.dram_tensor("idxk", [128, KT], i32,
                               kind="ExternalInput").ap(),
        "kvalid": nc.dram_tensor("kvalid", [128, KT], f32,
                                 kind="ExternalInput").ap(),
        "out": nc.dram_tensor("out", [ROWS * SEQ, HID], bf16,
                              kind="ExternalOutput").ap(),
    }
    with tile.TileContext(nc) as tc:
        _device_kernel(tc, aps)
    nc.compile()
    return nc


def _prepare_in_maps(inputs):
    input_ids = np.asarray(inputs["input_ids"], dtype=np.int32)
    token_type_ids = np.asarray(inputs["token_type_ids"], dtype=np.int32)
    word_emb = np.ascontiguousarray(
        np.asarray(inputs["word_emb"], np.float32).astype(BF16NP))
    pos_emb = np.asarray(inputs["pos_emb"], np.float32)
    tt_emb = np.asarray(inputs["tt_emb"], np.float32)
    entity_vec = np.ascontiguousarray(
        np.asarray(inputs["entityVec"], np.float32).astype(BF16NP))
    ke_w = np.asarray(inputs["ke_w"], np.float32)
    ke_b = np.asarray(inputs["ke_b"], np.float32)

    # fused side table: rows [tt*512 + pos] = pos_emb[pos] + tt_emb[tt],
    # second half additionally + ke_b (knowledge branch folds its bias in)
    base = (tt_emb[:, None, :] + pos_emb[None, :, :]).reshape(
        N_TYPES * MAX_POS, HID)
    table2 = np.ascontiguousarray(
        np.concatenate([base, base + ke_b[None, :]], axis=0).astype(BF16NP))
    ke_wT = np.ascontiguousarray(ke_w.T.astype(BF16NP))
    gamma_beta = np.ascontiguousarray(np.stack([
        np.asarray(inputs["w_gamma"], np.float32),
        np.asarray(inputs["w_beta"], np.float32),
        np.asarray(inputs["k_gamma"], np.float32),
        np.asarray(inputs["k_beta"], np.float32),
    ]).astype(BF16NP))

    wid, wtt, wpos, kid, ktt, kpos, kvalid = _compact(input_ids, token_type_ids)
    widx = wid.astype(np.int16)
    wtidx = (wpos + MAX_POS * wtt).astype(np.int16)
    kidx = kid.astype(np.int32)
    ktidx = (N_TYPES * MAX_POS + kpos + MAX_POS * ktt).astype(np.int16)
    kvf = kvalid.astype(np.float32)

    in_maps = []
    for c in range(NCORES):
        s = slice(c * ROWS, (c + 1) * ROWS)
        wflat = widx[s].reshape(WT * 128)       # token i of tile t at t*128+p
        tflat = wtidx[s].reshape(WT * 128)
        kflat = ktidx[s].reshape(KT * 128)
        idx16 = np.concatenate(
            [_wrap16(wflat[g * GRP * 128:(g + 1) * GRP * 128])
             for g in range(NG)]
            + [_wrap16(tflat[g * GRP * 128:(g + 1) * GRP * 128])
               for g in range(NG)]
            + [_wrap16(kflat)], axis=1)
        in_maps.append({
            "word_emb": word_emb,
            "entity_vec": entity_vec,
            "table2": table2,
            "ke_wT": ke_wT,
            "gamma_beta": gamma_beta,
            "idx16": np.ascontiguousarray(idx16),
            "idxk": np.ascontiguousarray(kidx[s].reshape(KT, 128).T),
            "kvalid": np.ascontiguousarray(kvf[s].reshape(KT, 128).T),
        })
    return in_maps


def run(inputs, trace=False):
    """Returns (full_output [64,320,768] f32, exec_time_ns or None)."""
    nc = build_program()
    in_maps = _prepare_in_maps(inputs)
    res = run_bass_kernel_spmd(nc, in_maps, list(range(NCORES)), trace=trace)
    out = np.concatenate(
        [np.asarray(r["out"]).astype(np.float32).reshape(ROWS, SEQ, HID)
         for r in res.results], axis=0)
    return out, res.exec_time_ns


def kernel(**inputs) -> np.ndarray:
    out, _ = run(inputs)
    return out


# revision 19
# speedup vs baseline: 1.3651x; 1.0830x over previous
"""Trainium2 Bass kernel for KnowledgeEmbeddings (ragged_sequence).

Contract: kernel(**inputs) takes FULL unsharded inputs (numpy), returns the
FULL [64, 320, 768] f32 output.  Internally shards batch rows over 8
NeuronCores (8 rows each), replicates embedding tables, and runs a Tile/Bass
kernel per core via run_bass_kernel_spmd.

V5: bf16 tables/compute (f32 upcast on host); word/table2 gathers via the
dma_gather ucode op (int16 indices), rotated across 4 SWDGE queues so
descriptor generation runs on different Q7 core pairs concurrently; tables
carry a precomputed row-sum column (row pitch 896 elems, 770-elem payload:
768 data + rowsum + pad), so each tile's mean falls out of the gathers and
the single fused add -- no reduction pass; LN sum-of-squares on ACT
(Square+accum); per-tile contiguous gamma/beta ops; batched strided writes.
"""

import functools
import numpy as np
import ml_dtypes

import concourse.bass as bass
import concourse.tile as tile
from concourse import bacc, mybir
from concourse.bass import IndirectOffsetOnAxis
from concourse.bass_utils import run_bass_kernel_spmd
from concourse.masks import make_identity

# Problem constants (hardcoded per spec nn_KnowledgeEmbeddings_80839874445880)
WORD_LEN = 256
KN_LEN = 64
VOCAB = 30522
N_ENT = 500000
HID = 768
MAX_POS = 512
N_TYPES = 2
D_ENT = 100
B = 64
SEQ = WORD_LEN + KN_LEN  # 320
EPS = 1e-12

NCORES = 8
ROWS = B // NCORES           # 8 batch rows per core
WT = ROWS * WORD_LEN // 128  # 16 word tiles per core
KT = ROWS * KN_LEN // 128    # 4 knowledge tiles per core
GRP = 4                      # tiles per gather/stats group
NG = WT // GRP               # word groups
GI = GRP * 128 // 16         # idx16 columns per group (32)
NI16 = (2 * NG + 1) * GI     # idx16 tensor columns (9 gathers x 32)

PITCH = 896                  # table row pitch (1792B, %256)
PAY = 896                    # gathered payload elems (1792B, %256 for dma_gather)
SUMC = 768                   # sum column index

f32 = mybir.dt.float32
bf16 = mybir.dt.bfloat16
i32 = mybir.dt.int32
i16 = mybir.dt.int16
AF = mybir.ActivationFunctionType
ALU = mybir.AluOpType

BF16NP = np.dtype(ml_dtypes.bfloat16)


# ---------------------------------------------------------------- host side

def _compact(ids: np.ndarray, tts: np.ndarray):
    """Vectorized numpy mirror of reference._compact_row."""
    ids = ids.astype(np.int64)
    wmask = (ids > 0) & (ids < VOCAB)
    worder = np.argsort(~wmask, axis=1, kind="stable")[:, :WORD_LEN]
    nw = wmask.sum(1, keepdims=True)
    wvalid = np.arange(WORD_LEN)[None, :] < nw
    wid = np.where(wvalid, np.take_along_axis(ids, worder, 1), 0)
    wtt = np.where(wvalid, np.take_along_axis(tts, worder, 1), 1)
    wpos = np.where(wvalid, worder, np.arange(WORD_LEN)[None, :])

    kmask = ids >= VOCAB
    korder = np.argsort(~kmask, axis=1, kind="stable")[:, :KN_LEN]
    nk = kmask.sum(1, keepdims=True)
    kvalid = np.arange(KN_LEN)[None, :] < nk
    kid = np.where(kvalid, np.take_along_axis(ids, korder, 1) - VOCAB, 0)
    ktt = np.where(kvalid, np.take_along_axis(tts, korder, 1), 0)
    kpos = np.where(kvalid, korder, 0)
    return wid, wtt, wpos, kid, ktt, kpos, kvalid


def _wrap16(flat: np.ndarray) -> np.ndarray:
    """[n] int16 -> [128, n//16] dma_gather index layout (value i at
    [i%16, i//16], replicated across the 8 Q7 16-partition blocks)."""
    n = flat.shape[0]
    w = flat.reshape(n // 16, 16).T  # [16, n//16]
    return np.tile(w, (8, 1))


def _aug(table_f32: np.ndarray) -> np.ndarray:
    """[R, 768] f32 -> [R, PITCH] bf16 with bf16 rowsum at col 768."""
    t = table_f32.astype(BF16NP)
    out = np.zeros((t.shape[0], PITCH), dtype=BF16NP)
    out[:, :HID] = t
    out[:, SUMC] = t.astype(np.float32).sum(axis=1).astype(BF16NP)
    return np.ascontiguousarray(out)


# ------------------------------------------------------------- device side

def _dma_gather(eng, out_ap, in_ap, idxs_ap, num_idxs, elem_size, queue_num):
    """bass.dma_gather minus the elem_size_bytes%256 assert (that constraint
    is transpose-only in the ucode; non-transpose descriptors take any size).
    Row pitch (in_ap.ap[0][0]) must still be a multiple of 128 elems."""
    from concourse import ap_utils
    assert in_ap.space == bass.MemorySpace.DRAM
    assert idxs_ap.space == bass.MemorySpace.SBUF
    assert out_ap.space == bass.MemorySpace.SBUF
    assert idxs_ap.dtype == i16
    assert in_ap.dtype == out_ap.dtype
    assert ap_utils.ap_is_contiguous(out_ap.ap[1:])
    assert ap_utils.ap_is_contiguous(idxs_ap.ap[1:])
    assert in_ap.ap[-1][1] == out_ap.ap[-1][1] == elem_size
    assert out_ap.ap[0][1] * out_ap.ap[1][1] == num_idxs
    stride_bytes = in_ap.ap[0][0] * mybir.dt.size(in_ap.dtype)
    assert stride_bytes % 256 == 0 and stride_bytes // 256 < 256
    _in_ap = eng.lower_ap_dma(in_ap, for_custom_bir_dma=True)
    return eng.add_instruction(
        mybir.InstDMAGatherAnt(
            name=eng.bass.get_next_instruction_name(),
            ins=[
                *_in_ap,
                eng.lower_ap(idxs_ap),
                eng.lower_val_access(eng.to_reg(num_idxs)),
            ],
            outs=[eng.lower_ap(out_ap)],
            transpose=False,
            num_idxs=num_idxs,
            elem_size=elem_size,
            stride_bytes_256=stride_bytes // 256,
            gen_mode=0,
            single_packet=True,
            queue_num=queue_num,
            sbuf_tokens_per_rank=0,
            sbuf_free_dim_per_rank=0,
            sbuf_free_dim_pad_per_rank=0,
            sbuf_byte_offset=0,
        )
    )


def _device_kernel(tc, aps):
    nc = tc.nc
    we, ev, tbl, kwT, gb, idx16, idxk, kvf, out = (
        aps["word_emb"], aps["entity_vec"], aps["table2"], aps["ke_wT"],
        aps["gamma_beta"], aps["idx16"], aps["idxk"], aps["kvalid"],
        aps["out"],
    )

    def tview(table_ap):
        return bass.AP(tensor=table_ap.tensor, offset=0,
                       ap=[list(table_ap.ap[0]), [1, PAY]])

    we_v, tbl_v = tview(we), tview(tbl)

    import contextlib
    with contextlib.ExitStack() as ctx:
        singles = ctx.enter_context(tc.tile_pool(name="singles", bufs=1))
        xpool = ctx.enter_context(tc.tile_pool(name="x", bufs=5))
        tpool = ctx.enter_context(tc.tile_pool(name="t", bufs=4))
        evpool = ctx.enter_context(tc.tile_pool(name="ev", bufs=2))
        spool = ctx.enter_context(tc.tile_pool(name="small", bufs=3))
        psum = ctx.enter_context(tc.tile_pool(name="psum", bufs=2, space="PSUM"))

        eps_sb = singles.tile([128, 1], f32)
        nc.vector.memset(eps_sb[:], EPS)

        # --- setup (once per core) ---
        idx16_sb = singles.tile([128, NI16], i16)
        nc.sync.dma_start(idx16_sb[:], idx16)
        idxk_sb = singles.tile([128, KT], i32)
        nc.sync.dma_start(idxk_sb[:], idxk)
        kv_sb = singles.tile([128, KT], f32)
        nc.sync.dma_start(kv_sb[:], kvf)
        kw_sb = singles.tile([128, PAY], bf16)
        nc.vector.memset(kw_sb[:], 0.0)
        nc.sync.dma_start(kw_sb[:D_ENT, :], kwT)
        ident = singles.tile([128, 128], bf16)
        make_identity(nc, ident[:])
        gbb = singles.tile([128, 4, HID], bf16)
        gb_bcast = bass.AP(tensor=gb.tensor, offset=gb.offset,
                           ap=[[0, 128]] + list(gb.ap))
        nc.gpsimd.dma_start(out=gbb[:], in_=gb_bcast)
        # two persistent transposed-EV buffers; rows D_ENT..127 stay zero
        EVTs = [singles.tile([128, 128], bf16, name=f"EVT{i}")
                for i in range(2)]
        for e in EVTs:
            nc.vector.memset(e[:], 0.0)

        def ln_finish(X, n, gi, SS_t, kv=None):
            """Normalize X [128, n, PAY] (bf16) in place given the rowsum in
            col SUMC and ACT-accumulated SS columns; gamma/beta rows
            2gi/2gi+1. Returns nothing; X[:, :, :HID] holds the output."""
            SM = bass.AP(tensor=X.tensor, offset=X.offset + SUMC,
                         ap=[list(X.ap[0]), [PAY, n]])
            # varH = SS - SM^2/HID ; rstd = 1/sqrt(varH/HID + eps)
            X1_t = spool.tile([128, GRP], f32, tag="X1")
            X1 = X1_t[:, :n]
            nc.vector.scalar_tensor_tensor(
                out=X1, in0=SM, scalar=1.0 / HID, in1=SM,
                op0=ALU.mult, op1=ALU.mult)
            VH_t = spool.tile([128, GRP], f32, tag="VH")
            VH = VH_t[:, :n]
            nc.vector.scalar_tensor_tensor(
                out=VH, in0=SS_t[:, :n], scalar=0.0, in1=X1,
                op0=ALU.add, op1=ALU.subtract)
            RS_t = spool.tile([128, GRP], f32, tag="RS")
            RS = RS_t[:, :n]
            nc.scalar.activation(RS, VH, func=AF.Sqrt, bias=eps_sb[:],
                                 scale=1.0 / HID)
            nc.vector.reciprocal(RS, RS)
            if kv is not None:
                nc.vector.tensor_mul(RS, RS, kv)
            # s1 = SM * rstd / HID  (so y = x*rstd - s1, per tile)
            S1_t = spool.tile([128, GRP], f32, tag="S1")
            S1 = S1_t[:, :n]
            nc.vector.scalar_tensor_tensor(
                out=S1, in0=SM, scalar=1.0 / HID, in1=RS,
                op0=ALU.mult, op1=ALU.mult)
            for i in range(n):
                Xi = X[:, i, :HID]
                nc.vector.tensor_scalar(
                    out=Xi, in0=Xi,
                    scalar1=RS_t[:, i:i + 1], scalar2=S1_t[:, i:i + 1],
                    op0=ALU.mult, op1=ALU.subtract,
                )
                nc.vector.tensor_mul(Xi, Xi, gbb[:, 2 * gi, :])
                nc.vector.tensor_add(Xi, Xi, gbb[:, 2 * gi + 1, :])

        # --- issue every gather up front.  At most 8 Pool-engine DMA
        # instructions total: the Tile scheduler has 8 DMASW sem lanes and
        # each lane is locked to the SWDGE queue of its first user, so with
        # <=8 Pool DMAs any queue assignment is conflict-free.  One big
        # gather per table: descgen streams descriptors to the SDMA rings,
        # so transfers overlap generation, and different queue_nums run
        # their descgen on different Q7 core pairs.
        A = [None] * NG
        T = [None] * NG
        for g in range(NG):
            A[g] = xpool.tile([128, GRP, PAY], bf16, tag="A", name=f"A{g}")
            _dma_gather(nc.gpsimd, A[g][:], we_v,
                        idx16_sb[:, g * GI:(g + 1) * GI],
                        GRP * 128, PAY, queue_num=0)
            T[g] = tpool.tile([128, GRP, PAY], bf16, tag="T", name=f"T{g}")
            _dma_gather(nc.gpsimd, T[g][:], tbl_v,
                        idx16_sb[:, (NG + g) * GI:(NG + g + 1) * GI],
                        GRP * 128, PAY, queue_num=0)
        Xk = singles.tile([128, KT, PAY], bf16)
        _dma_gather(nc.gpsimd, Xk[:], tbl_v,
                    idx16_sb[:, 2 * NG * GI:(2 * NG + 1) * GI],
                    KT * 128, PAY, queue_num=0)
        EV = evpool.tile([128, KT, D_ENT], bf16, tag="EV")
        for c in range(KT):
            nc.gpsimd.indirect_dma_start(
                out=EV[:, c, :], out_offset=None, in_=ev,
                in_offset=IndirectOffsetOnAxis(ap=idxk_sb[:, c:c + 1], axis=0),
            )

        # --- word tiles, in groups of GRP ---
        for g in range(NG):
            Ag = A[g][:]
            Tg = T[g][:]
            nc.vector.tensor_add(Ag[:, :, :SUMC + 2], Ag[:, :, :SUMC + 2],
                                 Tg[:, :, :SUMC + 2])
            SS_t = spool.tile([128, GRP], f32, tag="SS")
            scr = spool.tile([128, HID], bf16, tag="scr")
            for i in range(GRP):
                nc.scalar.activation(scr[:], Ag[:, i, :HID], func=AF.Square,
                                     accum_out=SS_t[:, i:i + 1])
            ln_finish(Ag, GRP, 0, SS_t)
            # tiles 4g..4g+3 = batch rows 2g, 2g+1, halves h=0/1
            for b in range(2):
                wout = bass.AP(
                    tensor=out.tensor, offset=(2 * g + b) * SEQ * HID,
                    ap=[[HID, 128], [128 * HID, 2], [1, HID]])
                nc.sync.dma_start(wout, Ag[:, 2 * b:2 * b + 2, :HID])

        # --- knowledge tiles (one group of KT) ---
        SS_t = spool.tile([128, GRP], f32, tag="SS")
        scr = spool.tile([128, HID], bf16, tag="scr")
        for c in range(KT):
            ps_t = psum.tile([D_ENT, 128], bf16, tag="pst")
            nc.tensor.transpose(out=ps_t[:], in_=EV[:, c, :], identity=ident[:])
            EVT = EVTs[c % 2]
            nc.scalar.copy(EVT[:D_ENT, :], ps_t[:])
            # halves [0:384) and [384:PAY) -- the second includes the sum col,
            # kw_sb's col 768 holds ke_w column sums so Xk's rowsum stays true
            for lo, hi in ((0, 384), (384, PAY)):
                pm = psum.tile([128, PAY - 384], f32, tag="mm")
                nc.tensor.matmul(
                    out=pm[:, :hi - lo], lhsT=EVT[:], rhs=kw_sb[:, lo:hi],
                    start=True, stop=True,
                )
                nc.vector.tensor_add(Xk[:, c, lo:hi], Xk[:, c, lo:hi],
                                     pm[:, :hi - lo])
            nc.scalar.activation(scr[:], Xk[:, c, :HID], func=AF.Square,
                                 accum_out=SS_t[:, c:c + 1])
        # rstd *= kvalid: pad rows then normalize to 0 -> output = k_beta
        ln_finish(Xk[:], KT, 1, SS_t, kv=kv_sb[:])
        for b2 in range(2):
            kout = bass.AP(
                tensor=out.tensor, offset=(WORD_LEN + b2 * SEQ) * HID,
                ap=[[HID, 64], [2 * SEQ * HID, KT], [1, HID]])
            nc.sync.dma_start(kout, Xk[64 * b2:64 * (b2 + 1), :, :HID])


@functools.lru_cache(maxsize=1)
def build_program():
    nc = bacc.Bacc("TRN2", target_bir_lowering=False, debug=False,
                   enable_asserts=False, num_swdge_queues=1)
    aps = {
        "word_emb": nc.dram_tensor("word_emb", [VOCAB, PITCH], bf16,
                                   kind="ExternalInput").ap(),
        "entity_vec": nc.dram_tensor("entity_vec", [N_ENT, D_ENT], bf16,
                                     kind="ExternalInput").ap(),
        "table2": nc.dram_tensor("table2", [2 * N_TYPES * MAX_POS, PITCH],
                                 bf16, kind="ExternalInput").ap(),
        "ke_wT": nc.dram_tensor("ke_wT", [D_ENT, PAY], bf16,
                                kind="ExternalInput").ap(),
        "gamma_beta": nc.dram_tensor("gamma_beta", [4, HID], bf16,
                                     kind="ExternalInput").ap(),
        "idx16": nc.dram_tensor("idx16", [128, NI16], i16,
                                kind="ExternalInput").ap(),
        "idxk": nc.dram_tensor("idxk", [128, KT], i32,
                               kind="ExternalInput").ap(),
        "kvalid": nc.dram_tensor("kvalid", [128, KT], f32,
                                 kind="ExternalInput").ap(),
        "out": nc.dram_tensor("out", [ROWS * SEQ, HID], bf16,
                              kind="ExternalOutput").ap(),
    }
    with tile.TileContext(nc) as tc:
        _device_kernel(tc, aps)
    nc.compile()
    return nc


def _prepare_in_maps(inputs):
    input_ids = np.asarray(inputs["input_ids"], dtype=np.int32)
    token_type_ids = np.asarray(inputs["token_type_ids"], dtype=np.int32)
    word_emb = _aug(np.asarray(inputs["word_emb"], np.float32))
    pos_emb = np.asarray(inputs["pos_emb"], np.float32)
    tt_emb = np.asarray(inputs["tt_emb"], np.float32)
    entity_vec = np.ascontiguousarray(
        np.asarray(inputs["entityVec"], np.float32).astype(BF16NP))
    ke_w = np.asarray(inputs["ke_w"], np.float32)
    ke_b = np.asarray(inputs["ke_b"], np.float32)

    # fused side table: rows [tt*512 + pos] = pos_emb[pos] + tt_emb[tt],
    # second half additionally + ke_b (knowledge branch folds its bias in)
    base = (tt_emb[:, None, :] + pos_emb[None, :, :]).reshape(
        N_TYPES * MAX_POS, HID)
    table2 = _aug(np.concatenate([base, base + ke_b[None, :]], axis=0))

    kwt = ke_w.T.astype(BF16NP)  # [D_ENT, HID]
    ke_wT = np.zeros((D_ENT, PAY), dtype=BF16NP)
    ke_wT[:, :HID] = kwt
    ke_wT[:, SUMC] = kwt.astype(np.float32).sum(axis=1).astype(BF16NP)
    ke_wT = np.ascontiguousarray(ke_wT)

    gamma_beta = np.ascontiguousarray(np.stack([
        np.asarray(inputs["w_gamma"], np.float32),
        np.asarray(inputs["w_beta"], np.float32),
        np.asarray(inputs["k_gamma"], np.float32),
        np.asarray(inputs["k_beta"], np.float32),
    ]).astype(BF16NP))

    wid, wtt, wpos, kid, ktt, kpos, kvalid = _compact(input_ids, token_type_ids)
    widx = wid.astype(np.int16)
    wtidx = (wpos + MAX_POS * wtt).astype(np.int16)
    kidx = kid.astype(np.int32)
    ktidx = (N_TYPES * MAX_POS + kpos + MAX_POS * ktt).astype(np.int16)
    kvf = kvalid.astype(np.float32)

    in_maps = []
    for c in range(NCORES):
        s = slice(c * ROWS, (c + 1) * ROWS)
        wflat = widx[s].reshape(WT * 128)       # token i of tile t at t*128+p
        tflat = wtidx[s].reshape(WT * 128)
        kflat = ktidx[s].reshape(KT * 128)
        idx16 = np.concatenate(
            [_wrap16(wflat[g * GRP * 128:(g + 1) * GRP * 128])
             for g in range(NG)]
            + [_wrap16(tflat[g * GRP * 128:(g + 1) * GRP * 128])
               for g in range(NG)]
            + [_wrap16(kflat)], axis=1)
        in_maps.append({
            "word_emb": word_emb,
            "entity_vec": entity_vec,
            "table2": table2,
            "ke_wT": ke_wT,
            "gamma_beta": gamma_beta,
            "idx16": np.ascontiguousarray(idx16),
            "idxk": np.ascontiguousarray(kidx[s].reshape(KT, 128).T),
            "kvalid": np.ascontiguousarray(kvf[s].reshape(KT, 128).T),
        })
    return in_maps


def run(inputs, trace=False):
    """Returns (full_output [64,320,768] f32, exec_time_ns or None)."""
    nc = build_program()
    in_maps = _prepare_in_maps(inputs)
    res = run_bass_kernel_spmd(nc, in_maps, list(range(NCORES)), trace=trace)
    out = np.concatenate(
        [np.asarray(r["out"]).astype(np.float32).reshape(ROWS, SEQ, HID)
         for r in res.results], axis=0)
    return out, res.exec_time_ns


def kernel(**inputs) -> np.ndarray:
    out, _ = run(inputs)
    return out


# revision 21
# speedup vs baseline: 1.3925x; 1.0201x over previous
"""Trainium2 Bass kernel for KnowledgeEmbeddings (ragged_sequence).

Contract: kernel(**inputs) takes FULL unsharded inputs (numpy), returns the
FULL [64, 320, 768] f32 output.  Internally shards batch rows over 8
NeuronCores (8 rows each), replicates embedding tables, and runs a Tile/Bass
kernel per core via run_bass_kernel_spmd.

V5: bf16 tables/compute (f32 upcast on host); word/table2 gathers via the
dma_gather ucode op (int16 indices), rotated across 4 SWDGE queues so
descriptor generation runs on different Q7 core pairs concurrently; tables
carry a precomputed row-sum column (row pitch 896 elems, 770-elem payload:
768 data + rowsum + pad), so each tile's mean falls out of the gathers and
the single fused add -- no reduction pass; LN sum-of-squares on ACT
(Square+accum); per-tile contiguous gamma/beta ops; batched strided writes.
"""

import functools
import numpy as np
import ml_dtypes

import concourse.bass as bass
import concourse.tile as tile
from concourse import bacc, mybir
from concourse.bass import IndirectOffsetOnAxis
from concourse.bass_utils import run_bass_kernel_spmd
from concourse.masks import make_identity

# Problem constants (hardcoded per spec nn_KnowledgeEmbeddings_80839874445880)
WORD_LEN = 256
KN_LEN = 64
VOCAB = 30522
N_ENT = 500000
HID = 768
MAX_POS = 512
N_TYPES = 2
D_ENT = 100
B = 64
SEQ = WORD_LEN + KN_LEN  # 320
EPS = 1e-12

NCORES = 8
ROWS = B // NCORES           # 8 batch rows per core
WT = ROWS * WORD_LEN // 128  # 16 word tiles per core
KT = ROWS * KN_LEN // 128    # 4 knowledge tiles per core
GRP = 4                      # tiles per gather/stats group
NG = WT // GRP               # word groups
GI = GRP * 128 // 16         # idx16 columns per group (32)
NI16 = (2 * NG + 1) * GI     # idx16 tensor columns (9 gathers x 32)

PITCH = 896                  # table row pitch (1792B, %256)
PAY = 896                    # gathered payload elems (1792B, %256 for dma_gather)
SUMC = 768                   # sum column index

f32 = mybir.dt.float32
bf16 = mybir.dt.bfloat16
i32 = mybir.dt.int32
i16 = mybir.dt.int16
AF = mybir.ActivationFunctionType
ALU = mybir.AluOpType

BF16NP = np.dtype(ml_dtypes.bfloat16)


# ---------------------------------------------------------------- host side

def _compact(ids: np.ndarray, tts: np.ndarray):
    """Vectorized numpy mirror of reference._compact_row."""
    ids = ids.astype(np.int64)
    wmask = (ids > 0) & (ids < VOCAB)
    worder = np.argsort(~wmask, axis=1, kind="stable")[:, :WORD_LEN]
    nw = wmask.sum(1, keepdims=True)
    wvalid = np.arange(WORD_LEN)[None, :] < nw
    wid = np.where(wvalid, np.take_along_axis(ids, worder, 1), 0)
    wtt = np.where(wvalid, np.take_along_axis(tts, worder, 1), 1)
    wpos = np.where(wvalid, worder, np.arange(WORD_LEN)[None, :])

    kmask = ids >= VOCAB
    korder = np.argsort(~kmask, axis=1, kind="stable")[:, :KN_LEN]
    nk = kmask.sum(1, keepdims=True)
    kvalid = np.arange(KN_LEN)[None, :] < nk
    kid = np.where(kvalid, np.take_along_axis(ids, korder, 1) - VOCAB, 0)
    ktt = np.where(kvalid, np.take_along_axis(tts, korder, 1), 0)
    kpos = np.where(kvalid, korder, 0)
    return wid, wtt, wpos, kid, ktt, kpos, kvalid


def _wrap16(flat: np.ndarray) -> np.ndarray:
    """[n] int16 -> [128, n//16] dma_gather index layout (value i at
    [i%16, i//16], replicated across the 8 Q7 16-partition blocks)."""
    n = flat.shape[0]
    w = flat.reshape(n // 16, 16).T  # [16, n//16]
    return np.tile(w, (8, 1))


def _aug(table_f32: np.ndarray) -> np.ndarray:
    """[R, 768] f32 -> [R, PITCH] bf16 with bf16 rowsum at col 768."""
    t = table_f32.astype(BF16NP)
    out = np.zeros((t.shape[0], PITCH), dtype=BF16NP)
    out[:, :HID] = t
    out[:, SUMC] = t.astype(np.float32).sum(axis=1).astype(BF16NP)
    return np.ascontiguousarray(out)


# ------------------------------------------------------------- device side

def _dma_gather(eng, out_ap, in_ap, idxs_ap, num_idxs, elem_size, queue_num):
    """bass.dma_gather minus the elem_size_bytes%256 assert (that constraint
    is transpose-only in the ucode; non-transpose descriptors take any size).
    Row pitch (in_ap.ap[0][0]) must still be a multiple of 128 elems."""
    from concourse import ap_utils
    assert in_ap.space == bass.MemorySpace.DRAM
    assert idxs_ap.space == bass.MemorySpace.SBUF
    assert out_ap.space == bass.MemorySpace.SBUF
    assert idxs_ap.dtype == i16
    assert in_ap.dtype == out_ap.dtype
    assert ap_utils.ap_is_contiguous(out_ap.ap[1:])
    assert ap_utils.ap_is_contiguous(idxs_ap.ap[1:])
    assert in_ap.ap[-1][1] == out_ap.ap[-1][1] == elem_size
    assert out_ap.ap[0][1] * out_ap.ap[1][1] == num_idxs
    stride_bytes = in_ap.ap[0][0] * mybir.dt.size(in_ap.dtype)
    assert stride_bytes % 256 == 0 and stride_bytes // 256 < 256
    _in_ap = eng.lower_ap_dma(in_ap, for_custom_bir_dma=True)
    return eng.add_instruction(
        mybir.InstDMAGatherAnt(
            name=eng.bass.get_next_instruction_name(),
            ins=[
                *_in_ap,
                eng.lower_ap(idxs_ap),
                eng.lower_val_access(eng.to_reg(num_idxs)),
            ],
            outs=[eng.lower_ap(out_ap)],
            transpose=False,
            num_idxs=num_idxs,
            elem_size=elem_size,
            stride_bytes_256=stride_bytes // 256,
            gen_mode=0,
            single_packet=True,
            queue_num=queue_num,
            sbuf_tokens_per_rank=0,
            sbuf_free_dim_per_rank=0,
            sbuf_free_dim_pad_per_rank=0,
            sbuf_byte_offset=0,
        )
    )


def _device_kernel(tc, aps):
    nc = tc.nc
    we, ev, tbl, kwT, gb, idx16, idxk, kvf, out = (
        aps["word_emb"], aps["entity_vec"], aps["table2"], aps["ke_wT"],
        aps["gamma_beta"], aps["idx16"], aps["idxk"], aps["kvalid"],
        aps["out"],
    )

    def tview(table_ap):
        return bass.AP(tensor=table_ap.tensor, offset=0,
                       ap=[list(table_ap.ap[0]), [1, PAY]])

    we_v, tbl_v = tview(we), tview(tbl)

    import contextlib
    with contextlib.ExitStack() as ctx:
        singles = ctx.enter_context(tc.tile_pool(name="singles", bufs=1))
        xpool = ctx.enter_context(tc.tile_pool(name="x", bufs=5))
        tpool = ctx.enter_context(tc.tile_pool(name="t", bufs=4))
        evpool = ctx.enter_context(tc.tile_pool(name="ev", bufs=2))
        spool = ctx.enter_context(tc.tile_pool(name="small", bufs=3))
        psum = ctx.enter_context(tc.tile_pool(name="psum", bufs=2, space="PSUM"))

        eps_sb = singles.tile([128, 1], f32)
        nc.vector.memset(eps_sb[:], EPS)

        # --- setup (once per core) ---
        idx16_sb = singles.tile([128, NI16], i16)
        nc.sync.dma_start(idx16_sb[:], idx16)
        idxk_sb = singles.tile([128, KT], i32)
        nc.sync.dma_start(idxk_sb[:], idxk)
        kv_sb = singles.tile([128, KT], f32)
        nc.sync.dma_start(kv_sb[:], kvf)
        kw_sb = singles.tile([128, PAY], bf16)
        nc.vector.memset(kw_sb[:], 0.0)
        nc.sync.dma_start(kw_sb[:D_ENT, :], kwT)
        ident = singles.tile([128, 128], bf16)
        make_identity(nc, ident[:])
        gbb = singles.tile([128, 4, HID], bf16)
        gb_bcast = bass.AP(tensor=gb.tensor, offset=gb.offset,
                           ap=[[0, 128]] + list(gb.ap))
        nc.gpsimd.dma_start(out=gbb[:], in_=gb_bcast)
        # two persistent transposed-EV buffers; rows D_ENT..127 stay zero
        EVTs = [singles.tile([128, 128], bf16, name=f"EVT{i}")
                for i in range(2)]
        for e in EVTs:
            nc.vector.memset(e[:], 0.0)

        def ln_finish(X, n, gi, SS_t, kv=None):
            """Normalize X [128, n, PAY] (bf16) in place given the rowsum in
            col SUMC and ACT-accumulated SS columns; gamma/beta rows
            2gi/2gi+1. Returns nothing; X[:, :, :HID] holds the output."""
            SM = bass.AP(tensor=X.tensor, offset=X.offset + SUMC,
                         ap=[list(X.ap[0]), [PAY, n]])
            # varH = SS - SM^2/HID ; rstd = 1/sqrt(varH/HID + eps)
            X1_t = spool.tile([128, GRP], f32, tag="X1")
            X1 = X1_t[:, :n]
            nc.vector.scalar_tensor_tensor(
                out=X1, in0=SM, scalar=1.0 / HID, in1=SM,
                op0=ALU.mult, op1=ALU.mult)
            VH_t = spool.tile([128, GRP], f32, tag="VH")
            VH = VH_t[:, :n]
            nc.vector.scalar_tensor_tensor(
                out=VH, in0=SS_t[:, :n], scalar=0.0, in1=X1,
                op0=ALU.add, op1=ALU.subtract)
            RS_t = spool.tile([128, GRP], f32, tag="RS")
            RS = RS_t[:, :n]
            nc.scalar.activation(RS, VH, func=AF.Sqrt, bias=eps_sb[:],
                                 scale=1.0 / HID)
            nc.vector.reciprocal(RS, RS)
            if kv is not None:
                nc.vector.tensor_mul(RS, RS, kv)
            # s1 = SM * rstd / HID  (so y = x*rstd - s1, per tile)
            S1_t = spool.tile([128, GRP], f32, tag="S1")
            S1 = S1_t[:, :n]
            nc.vector.scalar_tensor_tensor(
                out=S1, in0=SM, scalar=1.0 / HID, in1=RS,
                op0=ALU.mult, op1=ALU.mult)
            for i in range(n):
                Xi = X[:, i, :HID]
                nc.vector.tensor_scalar(
                    out=Xi, in0=Xi,
                    scalar1=RS_t[:, i:i + 1], scalar2=S1_t[:, i:i + 1],
                    op0=ALU.mult, op1=ALU.subtract,
                )
                nc.vector.tensor_mul(Xi, Xi, gbb[:, 2 * gi, :])
                nc.vector.tensor_add(Xi, Xi, gbb[:, 2 * gi + 1, :])

        # --- issue every gather up front.  At most 8 Pool-engine DMA
        # instructions total: the Tile scheduler has 8 DMASW sem lanes and
        # each lane is locked to the SWDGE queue of its first user, so with
        # <=8 Pool DMAs any queue assignment is conflict-free.  One big
        # gather per table: descgen streams descriptors to the SDMA rings,
        # so transfers overlap generation, and different queue_nums run
        # their descgen on different Q7 core pairs.
        # Pool-DMA issue order is load-bearing: 14 Pool DMAs cycle over the
        # 8 DMASW sem lanes (lane = position % 8, gbb broadcast = position 0)
        # and each lane is locked to the SWDGE queue of its users, so
        # instructions 8 apart must share a queue.  EV indirects and the gbb
        # broadcast are pinned to queue 0; the order below pairs them with
        # each other, leaving one gather (Xk) on queue 0 and the rest spread
        # over queues 1-3 (descgen runs on a different Q7 core pair each).
        A = [None] * NG
        T = [None] * NG
        EV = evpool.tile([128, KT, D_ENT], bf16, tag="EV")
        Xk = singles.tile([128, KT, PAY], bf16)

        def g_ev(c):
            nc.gpsimd.indirect_dma_start(
                out=EV[:, c, :], out_offset=None, in_=ev,
                in_offset=IndirectOffsetOnAxis(ap=idxk_sb[:, c:c + 1], axis=0),
            )

        def g_a(g, q):
            A[g] = xpool.tile([128, GRP, PAY], bf16, tag="A", name=f"A{g}")
            _dma_gather(nc.gpsimd, A[g][:], we_v,
                        idx16_sb[:, g * GI:(g + 1) * GI],
                        GRP * 128, PAY, queue_num=q)

        def g_t(g, q):
            T[g] = tpool.tile([128, GRP, PAY], bf16, tag="T", name=f"T{g}")
            _dma_gather(nc.gpsimd, T[g][:], tbl_v,
                        idx16_sb[:, (NG + g) * GI:(NG + g + 1) * GI],
                        GRP * 128, PAY, queue_num=q)

        # (the scheduler hoists the first dma_gather above leading EV
        # copies; final lane layout pairs gbb/EV/Xk on queue 0 and spreads
        # the word gathers over queues 1-3)
        g_ev(1)
        _dma_gather(nc.gpsimd, Xk[:], tbl_v,
                    idx16_sb[:, 2 * NG * GI:(2 * NG + 1) * GI],
                    KT * 128, PAY, queue_num=0)
        g_a(0, 1)
        g_t(0, 2)
        g_a(1, 3)
        g_t(1, 1)
        g_a(2, 2)
        g_ev(0)
        g_ev(2)
        g_ev(3)
        g_t(2, 1)
        g_a(3, 2)
        g_t(3, 3)

        # --- word tiles, in groups of GRP ---
        for g in range(NG):
            Ag = A[g][:]
            Tg = T[g][:]
            nc.vector.tensor_add(Ag[:, :, :SUMC + 2], Ag[:, :, :SUMC + 2],
                                 Tg[:, :, :SUMC + 2])
            SS_t = spool.tile([128, GRP], f32, tag="SS")
            scr = spool.tile([128, HID], bf16, tag="scr")
            for i in range(GRP):
                nc.scalar.activation(scr[:], Ag[:, i, :HID], func=AF.Square,
                                     accum_out=SS_t[:, i:i + 1])
            ln_finish(Ag, GRP, 0, SS_t)
            # tiles 4g..4g+3 = batch rows 2g, 2g+1, halves h=0/1
            for b in range(2):
                wout = bass.AP(
                    tensor=out.tensor, offset=(2 * g + b) * SEQ * HID,
                    ap=[[HID, 128], [128 * HID, 2], [1, HID]])
                nc.sync.dma_start(wout, Ag[:, 2 * b:2 * b + 2, :HID])

        # --- knowledge tiles (one group of KT) ---
        SS_t = spool.tile([128, GRP], f32, tag="SS")
        scr = spool.tile([128, HID], bf16, tag="scr")
        for c in range(KT):
            ps_t = psum.tile([D_ENT, 128], bf16, tag="pst")
            nc.tensor.transpose(out=ps_t[:], in_=EV[:, c, :], identity=ident[:])
            EVT = EVTs[c % 2]
            nc.scalar.copy(EVT[:D_ENT, :], ps_t[:])
            # halves [0:384) and [384:PAY) -- the second includes the sum col,
            # kw_sb's col 768 holds ke_w column sums so Xk's rowsum stays true
            for lo, hi in ((0, 384), (384, PAY)):
                pm = psum.tile([128, PAY - 384], f32, tag="mm")
                nc.tensor.matmul(
                    out=pm[:, :hi - lo], lhsT=EVT[:], rhs=kw_sb[:, lo:hi],
                    start=True, stop=True,
                )
                nc.vector.tensor_add(Xk[:, c, lo:hi], Xk[:, c, lo:hi],
                                     pm[:, :hi - lo])
            nc.scalar.activation(scr[:], Xk[:, c, :HID], func=AF.Square,
                                 accum_out=SS_t[:, c:c + 1])
        # rstd *= kvalid: pad rows then normalize to 0 -> output = k_beta
        ln_finish(Xk[:], KT, 1, SS_t, kv=kv_sb[:])
        for b2 in range(2):
            kout = bass.AP(
                tensor=out.tensor, offset=(WORD_LEN + b2 * SEQ) * HID,
                ap=[[HID, 64], [2 * SEQ * HID, KT], [1, HID]])
            nc.sync.dma_start(kout, Xk[64 * b2:64 * (b2 + 1), :, :HID])


@functools.lru_cache(maxsize=1)
def build_program():
    nc = bacc.Bacc("TRN2", target_bir_lowering=False, debug=False,
                   enable_asserts=False, num_swdge_queues=4)
    aps = {
        "word_emb": nc.dram_tensor("word_emb", [VOCAB, PITCH], bf16,
                                   kind="ExternalInput").ap(),
        "entity_vec": nc.dram_tensor("entity_vec", [N_ENT, D_ENT], bf16,
                                     kind="ExternalInput").ap(),
        "table2": nc.dram_tensor("table2", [2 * N_TYPES * MAX_POS, PITCH],
                                 bf16, kind="ExternalInput").ap(),
        "ke_wT": nc.dram_tensor("ke_wT", [D_ENT, PAY], bf16,
                                kind="ExternalInput").ap(),
        "gamma_beta": nc.dram_tensor("gamma_beta", [4, HID], bf16,
                                     kind="ExternalInput").ap(),
        "idx16": nc.dram_tensor("idx16", [128, NI16], i16,
                                kind="ExternalInput").ap(),
        "idxk": nc.dram_tensor("idxk", [128, KT], i32,
                               kind="ExternalInput").ap(),
        "kvalid": nc.dram_tensor("kvalid", [128, KT], f32,
                                 kind="ExternalInput").ap(),
        "out": nc.dram_tensor("out", [ROWS * SEQ, HID], bf16,
                              kind="ExternalOutput").ap(),
    }
    with tile.TileContext(nc) as tc:
        _device_kernel(tc, aps)
    nc.compile()
    return nc


def _prepare_in_maps(inputs):
    input_ids = np.asarray(inputs["input_ids"], dtype=np.int32)
    token_type_ids = np.asarray(inputs["token_type_ids"], dtype=np.int32)
    word_emb = _aug(np.asarray(inputs["word_emb"], np.float32))
    pos_emb = np.asarray(inputs["pos_emb"], np.float32)
    tt_emb = np.asarray(inputs["tt_emb"], np.float32)
    entity_vec = np.ascontiguousarray(
        np.asarray(inputs["entityVec"], np.float32).astype(BF16NP))
    ke_w = np.asarray(inputs["ke_w"], np.float32)
    ke_b = np.asarray(inputs["ke_b"], np.float32)

    # fused side table: rows [tt*512 + pos] = pos_emb[pos] + tt_emb[tt],
    # second half additionally + ke_b (knowledge branch folds its bias in)
    base = (tt_emb[:, None, :] + pos_emb[None, :, :]).reshape(
        N_TYPES * MAX_POS, HID)
    table2 = _aug(np.concatenate([base, base + ke_b[None, :]], axis=0))

    kwt = ke_w.T.astype(BF16NP)  # [D_ENT, HID]
    ke_wT = np.zeros((D_ENT, PAY), dtype=BF16NP)
    ke_wT[:, :HID] = kwt
    ke_wT[:, SUMC] = kwt.astype(np.float32).sum(axis=1).astype(BF16NP)
    ke_wT = np.ascontiguousarray(ke_wT)

    gamma_beta = np.ascontiguousarray(np.stack([
        np.asarray(inputs["w_gamma"], np.float32),
        np.asarray(inputs["w_beta"], np.float32),
        np.asarray(inputs["k_gamma"], np.float32),
        np.asarray(inputs["k_beta"], np.float32),
    ]).astype(BF16NP))

    wid, wtt, wpos, kid, ktt, kpos, kvalid = _compact(input_ids, token_type_ids)
    widx = wid.astype(np.int16)
    wtidx = (wpos + MAX_POS * wtt).astype(np.int16)
    kidx = kid.astype(np.int32)
    ktidx = (N_TYPES * MAX_POS + kpos + MAX_POS * ktt).astype(np.int16)
    kvf = kvalid.astype(np.float32)

    in_maps = []
    for c in range(NCORES):
        s = slice(c * ROWS, (c + 1) * ROWS)
        wflat = widx[s].reshape(WT * 128)       # token i of tile t at t*128+p
        tflat = wtidx[s].reshape(WT * 128)
        kflat = ktidx[s].reshape(KT * 128)
        idx16 = np.concatenate(
            [_wrap16(wflat[g * GRP * 128:(g + 1) * GRP * 128])
             for g in range(NG)]
            + [_wrap16(tflat[g * GRP * 128:(g + 1) * GRP * 128])
               for g in range(NG)]
            + [_wrap16(kflat)], axis=1)
        in_maps.append({
            "word_emb": word_emb,
            "entity_vec": entity_vec,
            "table2": table2,
            "ke_wT": ke_wT,
            "gamma_beta": gamma_beta,
            "idx16": np.ascontiguousarray(idx16),
            "idxk": np.ascontiguousarray(kidx[s].reshape(KT, 128).T),
            "kvalid": np.ascontiguousarray(kvf[s].reshape(KT, 128).T),
        })
    return in_maps


def run(inputs, trace=False):
    """Returns (full_output [64,320,768] f32, exec_time_ns or None)."""
    nc = build_program()
    in_maps = _prepare_in_maps(inputs)
    res = run_bass_kernel_spmd(nc, in_maps, list(range(NCORES)), trace=trace)
    out = np.concatenate(
        [np.asarray(r["out"]).astype(np.float32).reshape(ROWS, SEQ, HID)
         for r in res.results], axis=0)
    return out, res.exec_time_ns


def kernel(**inputs) -> np.ndarray:
    out, _ = run(inputs)
    return out
